# revision 38
# baseline (speedup 1.0000x reference)
"""Trainium2 Bass kernel for single-head attention.

reference:
  q = x @ Wq.T ; k = x @ Wk.T ; v = x @ Wv.T        (x: [B,S,D], W*: [D,D])
  out = softmax(q @ k.T / sqrt(D)) @ v              (B=4, S=4096, D=256)

Sharding: 8 cores = (batch b in 0..3) x (query-half h in 0..1).

All projections are folded into host-side prep (they are <2% of the FLOPs):
  G = Wq.T @ Wk  so  scores = (x @ G) @ x.T ;  Y = x @ G ;  V = x @ Wv.T
Each core receives fp16 tensors in DRAM layouts identical to the SBUF tiles
(so chunked DMAs match element-for-element with multi-KB descriptors):
  xs [128, 16c, 2dc, 2kc, 128k] = x^T  (scores stationary, keys of its batch)
  yt [128, 4j, 2dc, 512q]       = Y^T  (scores moving, its 2048 queries)
  vt [128k, 16c, 2kc, 256e]     = V    (AV stationary)
The device runs only the flash loop over 32 key chunks x 4 query tiles,
software-pipelined so the PE never waits on ACT:
  slot s:   S^T(s) = xs.T @ yt -> PSUM fp32 (4 matmuls)
            P^T(s) = exp(S^T/16 - ln8) -> fp16 (ACT);  pacc += P^T (DVE)
            O^T   += vt.T @ P^T(s-2)  (PE, accumulates in PSUM)
  per j:    den = ones.T @ pacc (PE) ; out = O^T * recip(den) (DVE) -> DMA.
Warm-up matmuls on memset tiles keep the PE busy (and its DVFS ramp running)
during the ~10us DMA lead-in.  The -ln8 bias keeps the fp16 softmax
denominator < 2^15 (it cancels in the normalization).  Core output is
O^T [256, 2048] stored tile-contiguous; the host reassembles.
"""

from contextlib import ExitStack

import numpy as np

B, S, D = 4, 4096, 256
H = S // 2          # queries per core
NCORE = 8
KC = S // 128       # 32 key chunks
QT = H // 512       # 4 query tiles
NSLOT = QT * KC // 2  # 64 pair-slots
LAG = 2             # AV runs this many slots behind scores
WARM = 8            # warm-up matmuls (512 rows each) during DMA lead-in
SCALE = 1.0 / np.sqrt(D)
PBIAS = -np.log(8.0)

_compiled_nc = None


def _build():
    import concourse.mybir as mybir
    import concourse.tile as tile
    from concourse import bacc

    F16 = mybir.dt.float16
    F32 = mybir.dt.float32
    EXP = mybir.ActivationFunctionType.Exp

    nc = bacc.Bacc("TRN2", target_bir_lowering=False, debug=False, num_devices=NCORE)
    # dram layouts identical to the SBUF tiles: chunk slices on both sides
    # match element-for-element, giving multi-KB contiguous DMA descriptors.
    # xs[p, c, a, b, f] = x^T[a*128+p, (2c+b)*128+f]   (a=dc half, b=kc in pair)
    # vt[p, c, b, e]    = V[(2c+b)*128+p, e]
    # yt[p, j, a, f]    = Y^T[a*128+p, j*512+f]
    xs_d = nc.dram_tensor("xs", [128, KC // 2, 2, 2, 128], F16, kind="ExternalInput")
    yt_d = nc.dram_tensor("yt", [128, QT, 2, 512], F16, kind="ExternalInput")
    vt_d = nc.dram_tensor("vt", [128, KC // 2, 2, 256], F16, kind="ExternalInput")
    # [j, p, ec, f]: per-partition rows are 2KB contiguous on the dram side
    ot = nc.dram_tensor("ot", [QT, 128, 2, 512], F16, kind="ExternalOutput")

    with tile.TileContext(nc) as tc, ExitStack() as ctx:
        const = ctx.enter_context(tc.tile_pool(name="const", bufs=1))
        big = ctx.enter_context(tc.tile_pool(name="big", bufs=1))
        pt_pool = ctx.enter_context(tc.tile_pool(name="ptp", bufs=6))
        small = ctx.enter_context(tc.tile_pool(name="small", bufs=3))

        # memsets split across gpsimd/vector so the warm-up matmuls (which
        # read ones/wmov) can start as soon as the engine preambles end
        ones = const.tile([128, 128], F16, name="ones")
        nc.gpsimd.memset(ones, 1.0)
        pbias = const.tile([128, 1], F32, name="pbias")
        nc.vector.memset(pbias, float(PBIAS))
        wmov = const.tile([128, 512], F16, name="wmov")
        nc.vector.memset(wmov, 0.5)

        xs = big.tile([128, KC // 2, 2, 2, 128], F16, name="xs")
        yt = big.tile([128, QT, 2, 512], F16, name="yt")
        vt = big.tile([128, KC // 2, 2, 256], F16, name="vt")
        osb = big.tile([128, QT, 2, 512], F16, name="osb")

        # input DMA: one tensor per queue, in consumption order, fine-grained
        # splits early (PE is hungry), coarse later (DMA runs ahead of compute)
        nc.scalar.dma_start(yt[:, 0:2, :, :], yt_d[:, 0:2, :, :])
        for c0, c1 in [(0, 1), (1, 2), (2, 4), (4, 8), (8, 16)]:
            nc.sync.dma_start(xs[:, c0:c1, :, :, :], xs_d[:, c0:c1, :, :, :])
            nc.gpsimd.dma_start(vt[:, c0:c1, :, :], vt_d[:, c0:c1, :, :])
        nc.scalar.dma_start(yt[:, 2:QT, :, :], yt_d[:, 2:QT, :, :])

        st_pool = ctx.enter_context(tc.tile_pool(name="st_psum", bufs=3, space="PSUM"))
        acc_pool = ctx.enter_context(tc.tile_pool(name="acc_psum", bufs=2, space="PSUM"))
        den_pool = ctx.enter_context(tc.tile_pool(name="den_psum", bufs=1, space="PSUM"))

        # warm-up: keep PE busy + DVFS ramping while inputs stream in
        warm = den_pool.tile([128, 512], F32, tag="den", name="warm")
        for w in range(WARM):
            nc.tensor.matmul(warm, ones, wmov, start=(w == 0), stop=(w == WARM - 1))

        # slot order: j0/j1 interleaved first (halves the per-chunk demand
        # rate while xs/vt stream in), then j2 at full rate; j3 runs as two
        # sequential 256-query half-tiles (column slices of the same PSUM
        # banks) so the first half's output DMA overlaps the second half's
        # compute -- the exposed tail is then only 128KB of writes.
        order = []
        for g in range(KC // 2):
            order += [(0, g, 0, 512), (1, g, 0, 512)]
        order += [(2, g, 0, 512) for g in range(KC // 2)]
        for q0 in (0, 256):
            order += [(3, g, q0, 256) for g in range(KC // 2)]

        pts = {}
        paccs = [None] * QT
        accs = [None] * QT

        def scores_slot(s):
            j, g, q0, w = order[s]
            pt = pt_pool.tile([128, 2, 512], F16, tag="pt", name=f"pt{s}")
            pts[s] = pt
            for u in range(2):
                st = st_pool.tile([128, 512], F32, tag="st", name=f"st{s}_{u}")
                nc.tensor.matmul(st[:, 0:w], xs[:, g, 0, u, :], yt[:, j, 0, q0:q0 + w], start=True, stop=False)
                nc.tensor.matmul(st[:, 0:w], xs[:, g, 1, u, :], yt[:, j, 1, q0:q0 + w], start=False, stop=True)
                nc.scalar.activation(pt[:, u, 0:w], st[:, 0:w], EXP, scale=float(SCALE), bias=pbias[:, :])
            # softmax denominator: accumulate exp tiles elementwise on DVE
            # (fp16 all-SBUF -> 2x mode); cross-partition sum via ones-matmul
            if g == 0 and q0 == 0:
                pacc = small.tile([128, 2, 512], F16, tag="pacc", name=f"pacc{j}")
                paccs[j] = pacc
            pacc = paccs[j]
            if g == 0:
                nc.vector.tensor_copy(pacc[:, :, q0:q0 + w], pt[:, :, 0:w])
            else:
                nc.vector.tensor_add(pacc[:, :, q0:q0 + w], pacc[:, :, q0:q0 + w], pt[:, :, 0:w])

        def av_slot(s):
            j, g, q0, w = order[s]
            if g == 0 and q0 == 0:
                accs[j] = (
                    acc_pool.tile([128, 512], F32, tag="ot0", name=f"ot0_{j}"),
                    acc_pool.tile([128, 512], F32, tag="ot1", name=f"ot1_{j}"),
                )
            ot0, ot1 = accs[j]
            pt = pts[s]
            for u in range(2):
                kc = g * 2 + u
                first, last = kc == 0, kc == KC - 1
                nc.tensor.matmul(ot0[:, q0:q0 + w], vt[:, g, u, 0:128], pt[:, u, 0:w], start=first, stop=last)
                nc.tensor.matmul(ot1[:, q0:q0 + w], vt[:, g, u, 128:256], pt[:, u, 0:w], start=first, stop=last)
            # j0/j1 finish as their last AV completes (acc banks are needed
            # for j2 two slots later); j2's finish is deferred into j3's AV
            # stream so the PE never waits on the pacc->den->recip chain
            if g == KC // 2 - 1:
                if j in (0, 1):
                    finish(j, 0, 512)
                elif j == 3:
                    finish(3, q0, 256)
            if j == 3 and q0 == 0 and g == 3:
                finish(2, 0, 512)

        def finish(j, q0, w):
            pacc = paccs[j]
            ot0, ot1 = accs[j]
            den = den_pool.tile([128, 512], F32, tag="den", name=f"den{j}_{q0}")
            for u in range(2):
                nc.tensor.matmul(den[:, q0:q0 + w], ones, pacc[:, u, q0:q0 + w],
                                 start=(u == 0), stop=(u == 1))
            rc = small.tile([128, 512], F32, tag="rc", name=f"rc{j}_{q0}")
            nc.vector.reciprocal_approx_fast(rc[:, q0:q0 + w], den[:, q0:q0 + w])
            for ec, acc in ((0, ot0), (1, ot1)):
                nc.vector.tensor_mul(osb[:, j, ec, q0:q0 + w], acc[:, q0:q0 + w], rc[:, q0:q0 + w])
            if j < QT - 1:
                (nc.sync if j % 2 == 0 else nc.gpsimd).dma_start(
                    ot[j], osb[:, j, :, :])
            else:
                # split each half-tile's write across the 3 DMA queues
                qs = [nc.sync, nc.gpsimd, nc.scalar]
                for k, p0 in enumerate(range(0, 128, 32)):
                    qs[k % 3].dma_start(ot[j, p0:p0 + 32, :, q0:q0 + w],
                                        osb[p0:p0 + 32, j, :, q0:q0 + w])

        nslot = len(order)
        for s in range(nslot + LAG):
            if s < nslot:
                scores_slot(s)
            if s >= LAG:
                av_slot(s - LAG)

    nc.compile()
    return nc


def _get_nc():
    global _compiled_nc
    if _compiled_nc is None:
        _compiled_nc = _build()
    return _compiled_nc


def make_in_maps(x, Wq, Wk, Wv):
    F16 = np.float16
    x = np.asarray(x, dtype=np.float32)
    G = (np.asarray(Wq, dtype=np.float64).T @ np.asarray(Wk, dtype=np.float64)).astype(np.float32)
    WvT = np.ascontiguousarray(np.asarray(Wv, dtype=np.float32).T)
    in_maps = [None] * NCORE
    for b in range(B):
        xb = x[b]                                  # [S, D]
        Y = (xb @ G).astype(F16)                   # [S, D] query-side
        V = (xb @ WvT).astype(F16)                 # [S, D]
        xT = np.ascontiguousarray(xb.T).astype(F16)  # [D, S]
        # xs_d [128, KC/2, 2dc, 2kc, 128]: [p,c,a,b,f] = xT[a*128+p, (2c+b)*128+f]
        xs = np.ascontiguousarray(
            xT.reshape(2, 128, KC // 2, 2, 128).transpose(1, 2, 0, 3, 4))
        # vt_d [128, KC/2, 2kc, 256]: [p,c,b,e] = V[(2c+b)*128+p, e]
        vt = np.ascontiguousarray(
            V.reshape(KC // 2, 2, 128, 256).transpose(2, 0, 1, 3))
        for h in range(2):
            Yh = Y[h * H:(h + 1) * H]              # [H, D]
            YhT = np.ascontiguousarray(Yh.T)       # [D, H]
            # yt_d [128, QT, 2dc, 512]: [p,j,a,f] = YhT[a*128+p, j*512+f]
            ytm = np.ascontiguousarray(
                YhT.reshape(2, 128, QT, 512).transpose(1, 2, 0, 3))
            in_maps[2 * b + h] = {"xs": xs, "yt": ytm, "vt": vt}
    return in_maps


def kernel(x, Wq, Wk, Wv):
    from concourse.bass_utils import run_bass_kernel_spmd

    nc = _get_nc()
    in_maps = make_in_maps(x, Wq, Wk, Wv)
    res = run_bass_kernel_spmd(nc, in_maps, core_ids=list(range(NCORE)))
    out = np.empty((B, S, D), dtype=np.float32)
    for c in range(NCORE):
        b, h = c // 2, c % 2
        # ot [QT, 128, 2ec, 512] fp16: out[q=j*512+f, e=ec*128+p]
        o = res.results[c]["ot"].astype(np.float32).transpose(0, 3, 2, 1).reshape(H, D)
        out[b, h * H:(h + 1) * H, :] = o
    return out


# revision 39
# speedup vs baseline: 1.1728x; 1.1728x over previous
"""Trainium2 Bass kernel for single-head attention.

reference:
  q = x @ Wq.T ; k = x @ Wk.T ; v = x @ Wv.T        (x: [B,S,D], W*: [D,D])
  out = softmax(q @ k.T / sqrt(D)) @ v              (B=4, S=4096, D=256)

Sharding: 8 cores = (batch b in 0..3) x (query-half h in 0..1).

All projections are folded into host-side prep (they are <2% of the FLOPs):
  G = Wq.T @ Wk  so  scores = (x @ G) @ x.T ;  Y = x @ G ;  V = x @ Wv.T
Each core receives fp16 tensors in DRAM layouts identical to the SBUF tiles
(so chunked DMAs match element-for-element with multi-KB descriptors):
  xs [128, 16c, 2dc, 2kc, 128k] = x^T  (scores stationary, keys of its batch)
  yt [128, 4j, 2dc, 512q]       = Y^T  (scores moving, its 2048 queries)
  vt [128k, 16c, 2kc, 256e]     = V    (AV stationary)
The device runs only the flash loop over 32 key chunks x 4 query tiles,
software-pipelined so the PE never waits on ACT:
  slot s:   S^T(s) = xs.T @ yt -> PSUM fp32 (4 matmuls)
            P^T(s) = exp(S^T/16 - ln8) -> fp16 (ACT);  pacc += P^T (DVE)
            O^T   += vt.T @ P^T(s-2)  (PE, accumulates in PSUM)
  per j:    den = ones.T @ pacc (PE) ; out = O^T * recip(den) (DVE) -> DMA.
Warm-up matmuls on memset tiles keep the PE busy (and its DVFS ramp running)
during the ~10us DMA lead-in.  The -ln8 bias keeps the fp16 softmax
denominator < 2^15 (it cancels in the normalization).  Core output is
O^T [256, 2048] stored tile-contiguous; the host reassembles.
"""

from contextlib import ExitStack

import numpy as np

B, S, D = 4, 4096, 256
H = S // 2          # queries per core
NCORE = 8
KC = S // 128       # 32 key chunks
QT = H // 512       # 4 query tiles
NSLOT = QT * KC // 2  # 64 pair-slots
LAG = 2             # AV runs this many slots behind scores
WARM = 8            # warm-up matmuls (512 rows each) during DMA lead-in
SCALE = 1.0 / np.sqrt(D)
PBIAS = -np.log(8.0)

_compiled_nc = None


def _build():
    import concourse.mybir as mybir
    import concourse.tile as tile
    from concourse import bacc

    F16 = mybir.dt.float16
    F32 = mybir.dt.float32
    EXP = mybir.ActivationFunctionType.Exp

    nc = bacc.Bacc("TRN2", target_bir_lowering=False, debug=False, num_devices=NCORE)
    # dram layouts identical to the SBUF tiles: chunk slices on both sides
    # match element-for-element, giving multi-KB contiguous DMA descriptors.
    # xs[p, c, a, b, f] = x^T[a*128+p, (2c+b)*128+f]   (a=dc half, b=kc in pair)
    # vt[p, c, b, e]    = V[(2c+b)*128+p, e]
    # yt[p, j, a, f]    = Y^T[a*128+p, j*512+f]
    xs_d = nc.dram_tensor("xs", [128, KC // 2, 2, 2, 128], F16, kind="ExternalInput")
    yt_d = nc.dram_tensor("yt", [128, QT, 2, 512], F16, kind="ExternalInput")
    vt_d = nc.dram_tensor("vt", [128, KC // 2, 2, 256], F16, kind="ExternalInput")
    # [j, p, ec, f]: per-partition rows are 2KB contiguous on the dram side
    ot = nc.dram_tensor("ot", [QT, 128, 2, 512], F16, kind="ExternalOutput")

    with tile.TileContext(nc) as tc, ExitStack() as ctx:
        const = ctx.enter_context(tc.tile_pool(name="const", bufs=1))
        big = ctx.enter_context(tc.tile_pool(name="big", bufs=1))
        pt_pool = ctx.enter_context(tc.tile_pool(name="ptp", bufs=6))
        small = ctx.enter_context(tc.tile_pool(name="small", bufs=3))

        # memsets split across gpsimd/vector so the warm-up matmuls (which
        # read ones/wmov) can start as soon as the engine preambles end
        ones = const.tile([128, 128], F16, name="ones")
        nc.gpsimd.memset(ones, 1.0)
        pbias = const.tile([128, 1], F32, name="pbias")
        nc.vector.memset(pbias, float(PBIAS))
        wmov = const.tile([128, 512], F16, name="wmov")
        nc.vector.memset(wmov, 0.5)

        xs = big.tile([128, KC // 2, 2, 2, 128], F16, name="xs")
        yt = big.tile([128, QT, 2, 512], F16, name="yt")
        vt = big.tile([128, KC // 2, 2, 256], F16, name="vt")
        osb = big.tile([128, QT, 2, 512], F16, name="osb")

        # input DMA: one tensor per queue, in consumption order, fine-grained
        # splits early (PE is hungry), coarse later (DMA runs ahead of compute)
        nc.scalar.dma_start(yt[:, 0:2, :, :], yt_d[:, 0:2, :, :])
        for c0, c1 in [(0, 1), (1, 2), (2, 3), (3, 4), (4, 8), (8, 16)]:
            nc.sync.dma_start(xs[:, c0:c1, :, :, :], xs_d[:, c0:c1, :, :, :])
            nc.gpsimd.dma_start(vt[:, c0:c1, :, :], vt_d[:, c0:c1, :, :])
        nc.scalar.dma_start(yt[:, 2:QT, :, :], yt_d[:, 2:QT, :, :])

        st_pool = ctx.enter_context(tc.tile_pool(name="st_psum", bufs=3, space="PSUM"))
        acc_pool = ctx.enter_context(tc.tile_pool(name="acc_psum", bufs=2, space="PSUM"))
        den_pool = ctx.enter_context(tc.tile_pool(name="den_psum", bufs=1, space="PSUM"))

        # warm-up: keep PE busy + DVFS ramping while inputs stream in
        warm = den_pool.tile([128, 512], F32, tag="den", name="warm")
        for w in range(WARM):
            nc.tensor.matmul(warm, ones, wmov, start=(w == 0), stop=(w == WARM - 1))

        # slot order: j0/j1 interleaved first (halves the per-chunk demand
        # rate while xs/vt stream in), then j2 at full rate; j3 runs as two
        # sequential 256-query half-tiles (column slices of the same PSUM
        # banks) so the first half's output DMA overlaps the second half's
        # compute -- the exposed tail is then only 128KB of writes.
        order = []
        for g in range(KC // 2):
            order += [(0, g, 0, 512), (1, g, 0, 512)]
        order += [(2, g, 0, 512) for g in range(KC // 2)]
        for q0 in (0, 256):
            order += [(3, g, q0, 256) for g in range(KC // 2)]

        pts = {}
        paccs = [None] * QT
        accs = [None] * QT

        def scores_slot(s):
            j, g, q0, w = order[s]
            pt = pt_pool.tile([128, 2, 512], F16, tag="pt", name=f"pt{s}")
            pts[s] = pt
            for u in range(2):
                st = st_pool.tile([128, 512], F32, tag="st", name=f"st{s}_{u}")
                nc.tensor.matmul(st[:, 0:w], xs[:, g, 0, u, :], yt[:, j, 0, q0:q0 + w], start=True, stop=False)
                nc.tensor.matmul(st[:, 0:w], xs[:, g, 1, u, :], yt[:, j, 1, q0:q0 + w], start=False, stop=True)
                nc.scalar.activation(pt[:, u, 0:w], st[:, 0:w], EXP, scale=float(SCALE), bias=pbias[:, :])
            # softmax denominator: accumulate exp tiles elementwise on DVE
            # (fp16 all-SBUF -> 2x mode); cross-partition sum via ones-matmul
            if g == 0 and q0 == 0:
                pacc = small.tile([128, 2, 512], F16, tag="pacc", name=f"pacc{j}")
                paccs[j] = pacc
            pacc = paccs[j]
            if g == 0:
                nc.vector.tensor_copy(pacc[:, :, q0:q0 + w], pt[:, :, 0:w])
            else:
                nc.vector.tensor_add(pacc[:, :, q0:q0 + w], pacc[:, :, q0:q0 + w], pt[:, :, 0:w])

        def av_slot(s):
            j, g, q0, w = order[s]
            if g == 0 and q0 == 0:
                accs[j] = (
                    acc_pool.tile([128, 512], F32, tag="ot0", name=f"ot0_{j}"),
                    acc_pool.tile([128, 512], F32, tag="ot1", name=f"ot1_{j}"),
                )
            ot0, ot1 = accs[j]
            pt = pts[s]
            for u in range(2):
                kc = g * 2 + u
                first, last = kc == 0, kc == KC - 1
                nc.tensor.matmul(ot0[:, q0:q0 + w], vt[:, g, u, 0:128], pt[:, u, 0:w], start=first, stop=last)
                nc.tensor.matmul(ot1[:, q0:q0 + w], vt[:, g, u, 128:256], pt[:, u, 0:w], start=first, stop=last)
            # j0/j1 finish as their last AV completes (acc banks are needed
            # for j2 two slots later); j2's finish is deferred into j3's AV
            # stream so the PE never waits on the pacc->den->recip chain
            if g == KC // 2 - 1:
                if j in (0, 1):
                    finish(j, 0, 512)
                elif j == 3:
                    finish(3, q0, 256)
            if j == 3 and q0 == 0 and g == 3:
                finish(2, 0, 512)

        def finish(j, q0, w):
            pacc = paccs[j]
            ot0, ot1 = accs[j]
            den = den_pool.tile([128, 512], F32, tag="den", name=f"den{j}_{q0}")
            for u in range(2):
                nc.tensor.matmul(den[:, q0:q0 + w], ones, pacc[:, u, q0:q0 + w],
                                 start=(u == 0), stop=(u == 1))
            rc = small.tile([128, 512], F32, tag="rc", name=f"rc{j}_{q0}")
            nc.vector.reciprocal_approx_fast(rc[:, q0:q0 + w], den[:, q0:q0 + w])
            for ec, acc in ((0, ot0), (1, ot1)):
                nc.vector.tensor_mul(osb[:, j, ec, q0:q0 + w], acc[:, q0:q0 + w], rc[:, q0:q0 + w])
            if j < QT - 1:
                (nc.sync if j % 2 == 0 else nc.gpsimd).dma_start(
                    ot[j], osb[:, j, :, :])
            else:
                # split each half-tile's write across the 3 DMA queues
                qs = [nc.sync, nc.gpsimd, nc.scalar]
                for k, p0 in enumerate(range(0, 128, 32)):
                    qs[k % 3].dma_start(ot[j, p0:p0 + 32, :, q0:q0 + w],
                                        osb[p0:p0 + 32, j, :, q0:q0 + w])

        nslot = len(order)
        for s in range(nslot + LAG):
            if s < nslot:
                scores_slot(s)
            if s >= LAG:
                av_slot(s - LAG)

    nc.compile()
    return nc


def _get_nc():
    global _compiled_nc
    if _compiled_nc is None:
        _compiled_nc = _build()
    return _compiled_nc


def make_in_maps(x, Wq, Wk, Wv):
    F16 = np.float16
    x = np.asarray(x, dtype=np.float32)
    G = (np.asarray(Wq, dtype=np.float64).T @ np.asarray(Wk, dtype=np.float64)).astype(np.float32)
    WvT = np.ascontiguousarray(np.asarray(Wv, dtype=np.float32).T)
    in_maps = [None] * NCORE
    for b in range(B):
        xb = x[b]                                  # [S, D]
        Y = (xb @ G).astype(F16)                   # [S, D] query-side
        V = (xb @ WvT).astype(F16)                 # [S, D]
        xT = np.ascontiguousarray(xb.T).astype(F16)  # [D, S]
        # xs_d [128, KC/2, 2dc, 2kc, 128]: [p,c,a,b,f] = xT[a*128+p, (2c+b)*128+f]
        xs = np.ascontiguousarray(
            xT.reshape(2, 128, KC // 2, 2, 128).transpose(1, 2, 0, 3, 4))
        # vt_d [128, KC/2, 2kc, 256]: [p,c,b,e] = V[(2c+b)*128+p, e]
        vt = np.ascontiguousarray(
            V.reshape(KC // 2, 2, 128, 256).transpose(2, 0, 1, 3))
        for h in range(2):
            Yh = Y[h * H:(h + 1) * H]              # [H, D]
            YhT = np.ascontiguousarray(Yh.T)       # [D, H]
            # yt_d [128, QT, 2dc, 512]: [p,j,a,f] = YhT[a*128+p, j*512+f]
            ytm = np.ascontiguousarray(
                YhT.reshape(2, 128, QT, 512).transpose(1, 2, 0, 3))
            in_maps[2 * b + h] = {"xs": xs, "yt": ytm, "vt": vt}
    return in_maps


def kernel(x, Wq, Wk, Wv):
    from concourse.bass_utils import run_bass_kernel_spmd

    nc = _get_nc()
    in_maps = make_in_maps(x, Wq, Wk, Wv)
    res = run_bass_kernel_spmd(nc, in_maps, core_ids=list(range(NCORE)))
    out = np.empty((B, S, D), dtype=np.float32)
    for c in range(NCORE):
        b, h = c // 2, c % 2
        # ot [QT, 128, 2ec, 512] fp16: out[q=j*512+f, e=ec*128+p]
        o = res.results[c]["ot"].astype(np.float32).transpose(0, 3, 2, 1).reshape(H, D)
        out[b, h * H:(h + 1) * H, :] = o
    return out
